# revision 20
# baseline (speedup 1.0000x reference)
"""Trainium2 Bass kernel for nn_Net_18966575579675 (dense_mlp).

722 independent tiny MLPs: per net n, per batch row b:
  x = [prior[b,n], camera[b,:]]            (11)
  h1 = relu(x @ W1[n] + b1[n])             (10)
  h2 = relu(h1 @ W2[n] + b2[n])            (10)
  out[b,n,:] = h2 @ W3[n] + b3[n]          (6)

Strategy: data-parallel over batch (8 cores x 1024 rows). Per core, nets are
processed in 64 groups of 12 as block-diagonal bf16 matmuls on the full PE
array (serial LDW+MM pipeline — measured faster than 32x32 array tiling,
whose per-MM ldweights serialize on TRN2):
  L1: K=23 rhs = [camera(10); prior-of-group(12); ones], lhsT [23,121]
      (camera rows dense, prior rows diagonal, ones row carries b1;
      M col 120 is a ones-passthrough so h1 row 120 == 1).
  L2: K=121 rhs = h1 (120 rows + ones), lhsT [121,121] blockdiag + bias row.
  L3: K=121, lhsT [121,72], biases on the ones row.
Relu/copy is fused into the PSUM->SBUF eviction (DVE / ACT alternating).
Output leaves the device as [group*72 + 6*n' + l, batch]; the host
transposes during the unshard step. Groups are software-pipelined
(L1(g) | L2(g-1) | L3(g-2)) so evictions overlap the PE stream.
"""

import sys

sys.path.insert(0, "/opt/trn_rl_repo")

import numpy as np
import ml_dtypes

import concourse.bass as bass
import concourse.bacc as bacc
import concourse.mybir as mybir
from concourse import tile
from concourse.tile import add_dep_helper

F32 = mybir.dt.float32
BF16 = mybir.dt.bfloat16

# Problem constants
B = 8192
N_NETS = 722
CAM = 10
H = 10
LOD = 6
N_CORES = 8

# Kernel geometry (full-size)
BC = B // N_CORES      # batch per core (1024)
NG = 64                # groups of 12 nets (64*12 = 768, padded)
NPAD = NG * 12


def _pad_nets(a, npad):
    """Pad (or slice) axis 0 (net axis) with zeros up to npad."""
    if npad <= a.shape[0]:
        return a[:npad]
    pad = [(0, npad - a.shape[0])] + [(0, 0)] * (a.ndim - 1)
    return np.pad(a, pad)


def build_host_tensors(prior_lod, camera, W1, b1, W2, b2, W3, b3,
                       bc=BC, ng=NG, n_cores=N_CORES):
    """Build per-core DRAM input tensors (numpy). Returns list of dicts."""
    npad = ng * 12
    gs = ng // 4            # group-sets (groups per row-group slot)

    W1 = _pad_nets(np.asarray(W1, np.float32), npad)
    b1 = _pad_nets(np.asarray(b1, np.float32), npad)
    W2 = _pad_nets(np.asarray(W2, np.float32), npad)
    b2 = _pad_nets(np.asarray(b2, np.float32), npad)
    W3 = _pad_nets(np.asarray(W3, np.float32), npad)
    b3 = _pad_nets(np.asarray(b3, np.float32), npad)
    prior = np.asarray(prior_lod, np.float32)
    camera = np.asarray(camera, np.float32)

    # ---- weights (shared by all cores) ----
    # W1H[g, k, m]: K=121 (only rows 32i..32i+22 nonzero, i = g%4), M=121
    W1H = np.zeros((ng, 121, 121), np.float32)
    for g in range(ng):
        i = g % 4
        r = 32 * i
        for n_ in range(12):
            n = 12 * g + n_
            W1H[g, r:r + 10, 10 * n_:10 * n_ + 10] = W1[n, 1:, :]
            W1H[g, r + 10 + n_, 10 * n_:10 * n_ + 10] = W1[n, 0, :]
            W1H[g, r + 22, 10 * n_:10 * n_ + 10] = b1[n]
        W1H[g, r + 22, 120] = 1.0   # ones-passthrough
    W1H = np.ascontiguousarray(
        W1H.transpose(1, 0, 2).reshape(121, ng * 121)).astype(
        ml_dtypes.bfloat16)

    # W2H[g, k, m]: K=121, M=121
    W2H = np.zeros((ng, 121, 121), np.float32)
    for g in range(ng):
        for n_ in range(12):
            n = 12 * g + n_
            W2H[g, 10 * n_:10 * n_ + 10, 10 * n_:10 * n_ + 10] = W2[n]
            W2H[g, 120, 10 * n_:10 * n_ + 10] = b2[n]
        W2H[g, 120, 120] = 1.0
    W2H = np.ascontiguousarray(
        W2H.transpose(1, 0, 2).reshape(121, ng * 121)).astype(
        ml_dtypes.bfloat16)

    # W3H[g, k, m]: K=121, M=72
    W3H = np.zeros((ng, 121, 72), np.float32)
    for g in range(ng):
        for n_ in range(12):
            n = 12 * g + n_
            W3H[g, 10 * n_:10 * n_ + 10, 6 * n_:6 * n_ + 6] = W3[n]
            W3H[g, 120, 6 * n_:6 * n_ + 6] = b3[n]
    W3H = np.ascontiguousarray(
        W3H.transpose(1, 0, 2).reshape(121, ng * 72)).astype(
        ml_dtypes.bfloat16)

    # ---- per-core xin: [4, 23, gs, bc] ----
    in_maps = []
    for core in range(n_cores):
        b0 = core * bc
        pr = prior[b0:b0 + bc]                  # [bc, 722]
        cam = camera[b0:b0 + bc]                # [bc, 10]
        xin = np.zeros((4, 23, gs, bc), np.float32)
        xin[:, 0:10] = cam.T[None, :, None, :]
        xin[:, 22] = 1.0
        for i in range(4):
            for s in range(gs):
                g = 4 * s + i
                nets = 12 * g + np.arange(12)
                valid = nets < N_NETS
                rows = np.zeros((12, bc), np.float32)
                rows[valid] = pr[:, nets[valid]].T
                xin[i, 10:22, s] = rows
        xin = np.ascontiguousarray(xin.reshape(4, 23, gs * bc)).astype(
            ml_dtypes.bfloat16)
        in_maps.append({"xin": xin, "w1": W1H, "w2": W2H, "w3": W3H})
    return in_maps


def build_program(bc=BC, ng=NG, reps=0):
    """Build the per-core Bass program (SPMD; identical on all cores).

    reps>0 wraps the whole body in a For_i repeat loop (timing builds only).
    """
    gs = ng // 4
    hc = bc // 2   # chunk size (matmul free dim)

    nc = bacc.Bacc(None)
    xin_d = nc.dram_tensor("xin", [4, 23, gs * bc], BF16, kind="ExternalInput")
    w1_d = nc.dram_tensor("w1", [121, ng * 121], BF16, kind="ExternalInput")
    w2_d = nc.dram_tensor("w2", [121, ng * 121], BF16, kind="ExternalInput")
    w3_d = nc.dram_tensor("w3", [121, ng * 72], BF16, kind="ExternalInput")
    ot_d = nc.dram_tensor("OT", [ng * 72, bc], F32, kind="ExternalOutput")

    import contextlib
    with tile.TileContext(nc) as tc:
        with tc.tile_pool(name="fix", bufs=1) as fix, \
             tc.tile_pool(name="psum", bufs=4, space="PSUM") as pp, \
             (tc.For_i(0, reps, 1) if reps else contextlib.nullcontext()):
            X = fix.tile([128, gs * bc], BF16, tag="X")
            W1s = fix.tile([128, ng * 121], BF16, tag="W1s")
            W2s = fix.tile([128, ng * 121], BF16, tag="W2s")
            W3s = fix.tile([128, ng * 72], BF16, tag="W3s")
            h1 = [fix.tile([128, bc], BF16, tag=f"h1{x}", name=f"h1{x}")
                  for x in "abc"]
            h2 = [fix.tile([128, bc], BF16, tag=f"h2{x}", name=f"h2{x}")
                  for x in "abc"]
            osb = [fix.tile([128, bc], F32, tag=f"osb{x}", name=f"osb{x}")
                   for x in "abc"]

            # ---- input DMAs ----
            nc.gpsimd.memset(X[:, :], 0)
            for i in range(4):
                nc.sync.dma_start(out=X[32 * i:32 * i + 23, :], in_=xin_d[i])
            nc.sync.dma_start(out=W1s[0:121, :], in_=w1_d[:])
            nc.sync.dma_start(out=W2s[0:121, :], in_=w2_d[:])
            nc.sync.dma_start(out=W3s[0:121, :], in_=w3_d[:])

            pe_prev = [None]

            def chain(mm):
                if pe_prev[0] is not None:
                    add_dep_helper(mm.ins, pe_prev[0], reason="pe-order")
                pe_prev[0] = mm.ins

            def l1(g):
                s = g // 4
                Tt = pp.tile([128, bc], F32, tag="ps", name="psL1")
                for c in range(2):
                    chain(nc.tensor.matmul(
                        Tt[0:121, c * hc:(c + 1) * hc],
                        W1s[0:121, g * 121:(g + 1) * 121],
                        X[0:121, s * bc + c * hc:s * bc + (c + 1) * hc],
                        tile_position=(0, 0),
                    ))
                return Tt

            def l23(g, W, wid, src, M):
                Tt = pp.tile([128, bc], F32, tag="ps", name="psL23")
                for c in range(2):
                    chain(nc.tensor.matmul(
                        Tt[0:M, c * hc:(c + 1) * hc],
                        W[0:121, g * wid:g * wid + M],
                        src[0:121, c * hc:(c + 1) * hc],
                        tile_position=(0, 0),
                    ))
                return Tt

            def evict(Tt, dst, rows, eng, relu):
                src = Tt[0:rows, :]
                d = dst[0:rows, :]
                if eng == 0:
                    if relu:
                        nc.vector.tensor_scalar_max(d, src, 0.0)
                    else:
                        nc.vector.tensor_scalar_add(d, src, 0.0)
                else:
                    f = (mybir.ActivationFunctionType.Relu if relu
                         else mybir.ActivationFunctionType.Copy)
                    nc.scalar.activation(d, src, f)

            # software pipeline: L1(g) | L2(g-D2) | L3(g-2*D2)
            import os
            D2 = int(os.environ.get("PIPE_D", "2"))
            eng = 0
            for gg in range(ng + 2 * D2):
                if gg < ng:
                    T1 = l1(gg)
                    evict(T1, h1[gg % 3], 121, eng, True)
                    eng ^= 1
                if D2 <= gg < ng + D2:
                    g2 = gg - D2
                    T2 = l23(g2, W2s, 121, h1[g2 % 3], 121)
                    evict(T2, h2[g2 % 3], 121, eng, True)
                    eng ^= 1
                if 2 * D2 <= gg:
                    g3 = gg - 2 * D2
                    T3 = l23(g3, W3s, 72, h2[g3 % 3], 72)
                    evict(T3, osb[g3 % 3], 72, eng, False)
                    eng ^= 1
                    nc.sync.dma_start(
                        out=ot_d[g3 * 72:(g3 + 1) * 72, :],
                        in_=osb[g3 % 3][0:72, :])
    nc.finalize()
    return nc


def _unshard(results, bc=BC, ng=NG):
    out = np.empty((N_CORES * bc, N_NETS, LOD), np.float32)
    for core, res in enumerate(results):
        ot = np.asarray(res["OT"], np.float32)           # [ng*72, bc]
        o = ot.reshape(ng * 12, LOD, bc)                 # [net, l, b]
        o = o.transpose(2, 0, 1)                         # [b, net, l]
        out[core * bc:(core + 1) * bc] = o[:, :N_NETS, :]
    return out


_PROGRAM_CACHE = {}


def kernel(prior_lod, camera, W1, b1, W2, b2, W3, b3):
    from concourse.bass_utils import run_bass_kernel_spmd
    in_maps = build_host_tensors(prior_lod, camera, W1, b1, W2, b2, W3, b3)
    key = (BC, NG)
    if key not in _PROGRAM_CACHE:
        _PROGRAM_CACHE[key] = build_program()
    nc = _PROGRAM_CACHE[key]
    res = run_bass_kernel_spmd(nc, in_maps, list(range(N_CORES)))
    return _unshard(res.results)
